# revision 8
# baseline (speedup 1.0000x reference)
"""Trainium2 Bass kernel for a dense transformer block (nn_Block_7911329760080).

Reference computation (B=2, T=2048 tokens, C=1024 channels, 16 heads, fp32):
    x = x + Attn(LN1(x));  x = x + MLP(LN2(x))   [full non-causal attention]

Sharding: token-parallel over 8 cores (4 cores per batch, 512 tokens each).
Each core recomputes LN1+K+V for its whole batch (no collectives); Q /
attention / MLP only for its own 512-token slice.

Layout: activations are kept feature-major ([feature, token], "T" suffix) so
every matmul (out = lhsT.T @ rhs, contraction on partitions) directly produces
the layout the next one consumes — zero on-device transposes.  LN statistics
come from ones-vector matmuls; softmax is max-free (scores are small) with the
per-query normalizer from an interleaved ones-column in V, produced for free
by the P@V matmul.  All matmuls run in float32r (full PE rate at N>=256).
"""

import numpy as np
import sys
from contextlib import ExitStack

sys.path.insert(0, "/opt/trn_rl_repo/concourse")
sys.path.insert(0, "/opt/trn_rl_repo")

import concourse.bacc as bacc
import concourse.mybir as mybir
import concourse.tile as tile

F32 = mybir.dt.float32
F32R = mybir.dt.float32r
ACTF = mybir.ActivationFunctionType

N_CORES = 8
B, T, C = 2, 2048, 1024
NH, HD = 16, 64
TQ = T * B // N_CORES          # 512 tokens per core
HID = 4 * C                    # 4096
NCT = C // 128                 # 8 c-tiles
NHT = HID // 128               # 32 hidden-dim tiles
NTT = T // 128                 # 16 token-tiles (full batch)
LN_EPS = 1e-5

# colpack column layout ([128, n] per-partition bias/scale columns)
CP_BQ8, CP_BK, CP_BO, CP_B2, CP_G1, CP_BL1, CP_G2, CP_BL2, CP_BV = (
    0, 8, 16, 24, 32, 40, 48, 56, 64)
CP_B1 = 72          # 32 cols
CP_EPS, CP_NEG1, CP_QSCL = 104, 105, 106
CP_N = 107

_CACHE = {}


def _pack_cols(vec):
    """[n*128] -> [128, n]: column j holds vec[128j:128j+128]."""
    return np.ascontiguousarray(vec.reshape(-1, 128).T)


def _grid(w):
    """[K, F] -> [K/128, F/128, 128, 128] contiguous tile grid."""
    K, F = w.shape
    return np.ascontiguousarray(
        w.reshape(K // 128, 128, F // 128, 128).transpose(0, 2, 1, 3))


def _build_program():
    nc = bacc.Bacc("TRN2", target_bir_lowering=False, debug=False,
                   num_devices=N_CORES)

    def din(name, shape):
        return nc.dram_tensor(name, list(shape), F32, kind="ExternalInput")

    xbT = din("xbT", (C, T))            # full batch, feature-major
    xqT = din("xqT", (C, TQ))           # own token slice, feature-major
    wq_g = din("wq_g", (NCT, NCT, 128, 128))
    wk_g = din("wk_g", (NCT, NCT, 128, 128))
    wv = din("wv", (C, C))              # natural layout (used as rhs)
    wo_g = din("wo_g", (NCT, NCT, 128, 128))
    w1_g = din("w1_g", (NCT, NHT, 128, 128))
    w2_g = din("w2_g", (NHT, NCT, 128, 128))
    colpack = din("colpack", (128, CP_N))
    out_d = nc.dram_tensor("outT", [C, TQ], F32, kind="ExternalOutput")

    with tile.TileContext(nc) as tc, ExitStack() as top:
        consts = top.enter_context(tc.tile_pool(name="consts", bufs=1))
        # x2T straddles the attention-pool close boundary -> own early pool
        p2x = top.enter_context(tc.tile_pool(name="p2x", bufs=1))

        cp = consts.tile([128, CP_N], F32)
        nc.sync.dma_start(out=cp, in_=colpack.ap())
        ones_col = consts.tile([128, 1], F32R)
        nc.vector.memset(ones_col.bitcast(F32), 1.0)
        ones_row = consts.tile([1, 128], F32R)
        nc.vector.memset(ones_row.bitcast(F32), 1.0)

        def col(idx):
            return cp[:, idx:idx + 1]

        def row_const(idx):
            return cp[0:1, idx:idx + 1]

        # ---------------- LayerNorm (feature-major) ----------------
        def layernorm(src, Tn, g_idx, b_idx, out_pool, out_tag, st):
            """src: DRAM tensor [C, Tn] (streamed twice) or list of NCT SBUF
            tiles [128, Tn] F32R.  Writes NCT tiles [128, Tn] F32R holding
            LN(src)*g + b into out_pool; returns them."""
            from_dram = not isinstance(src, list)
            nch = Tn // 512
            ps_st = st.enter_context(
                tc.tile_pool(name=f"lnp_{out_tag}", bufs=1, space="PSUM"))
            ps_bc = st.enter_context(
                tc.tile_pool(name=f"lnpb_{out_tag}", bufs=1, space="PSUM"))
            stream = st.enter_context(tc.tile_pool(name=f"lns_{out_tag}", bufs=3))
            work = st.enter_context(tc.tile_pool(name=f"lnw_{out_tag}", bufs=2))
            rows = st.enter_context(tc.tile_pool(name=f"lnr_{out_tag}", bufs=1))
            bc_pool = st.enter_context(tc.tile_pool(name=f"lnb_{out_tag}", bufs=1))

            a_bc = bc_pool.tile([128, Tn], F32, tag="a_bc")
            c_bc = bc_pool.tile([128, Tn], F32, tag="c_bc")

            def src_chunk(ct, sl):
                if from_dram:
                    xc = stream.tile([128, 512], F32R, tag="xc")
                    nc.sync.dma_start(
                        out=xc,
                        in_=src.ap()[ct * 128:(ct + 1) * 128, sl].bitcast(F32R))
                    return xc
                return src[ct][:, sl]

            for ch in range(nch):
                sl = slice(ch * 512, ch * 512 + 512)
                ps_s = ps_st.tile([1, 512], F32, tag="ps_s")
                ps_q = ps_st.tile([1, 512], F32, tag="ps_q")
                for ct in range(NCT):
                    xc = src_chunk(ct, sl)
                    nc.tensor.matmul(ps_s, ones_col, xc,
                                     start=(ct == 0), stop=(ct == NCT - 1))
                    sq = work.tile([128, 512], F32R, tag="sq")
                    nc.scalar.activation(sq, xc.bitcast(F32), ACTF.Square)
                    nc.tensor.matmul(ps_q, ones_col, sq,
                                     start=(ct == 0), stop=(ct == NCT - 1))
                # per-chunk row math on [1, 512]
                mu = rows.tile([1, 512], F32, tag="mu")
                nc.vector.tensor_scalar_mul(mu, ps_s, 1.0 / C)
                msq = rows.tile([1, 512], F32, tag="msq")
                nc.vector.tensor_scalar_mul(msq, ps_q, 1.0 / C)
                mu2 = rows.tile([1, 512], F32, tag="mu2")
                nc.vector.tensor_mul(mu2, mu, mu)
                var = rows.tile([1, 512], F32, tag="var")
                nc.vector.tensor_sub(var, msq, mu2)
                std = rows.tile([1, 512], F32, tag="std")
                nc.scalar.activation(std, var, ACTF.Sqrt, bias=row_const(CP_EPS))
                rstd = rows.tile([1, 512], F32, tag="rstd")
                nc.vector.reciprocal(rstd, std)
                c0 = rows.tile([1, 512], F32, tag="c0")
                nc.vector.tensor_mul(c0, mu, rstd)
                rstd_r = rows.tile([1, 512], F32R, tag="rstd_r")
                nc.scalar.activation(rstd_r, rstd, ACTF.Copy)
                nmu_r = rows.tile([1, 512], F32R, tag="nmu_r")
                nc.scalar.activation(nmu_r, c0, ACTF.Copy, scale=row_const(CP_NEG1))
                ps_a = ps_bc.tile([128, 512], F32, tag="ps_a")
                nc.tensor.matmul(ps_a, ones_row, rstd_r, start=True, stop=True)
                nc.vector.tensor_copy(a_bc[:, sl], ps_a)
                ps_c = ps_bc.tile([128, 512], F32, tag="ps_c")
                nc.tensor.matmul(ps_c, ones_row, nmu_r, start=True, stop=True)
                nc.vector.tensor_copy(c_bc[:, sl], ps_c)

            outs = []
            for ct in range(NCT):
                o = out_pool.tile([128, Tn], F32R, tag=f"{out_tag}{ct}")
                outs.append(o)
                for ch in range(nch):
                    sl = slice(ch * 512, ch * 512 + 512)
                    xc = src_chunk(ct, sl)
                    t1 = work.tile([128, 512], F32, tag="t1")
                    nc.vector.tensor_mul(t1, xc.bitcast(F32), a_bc[:, sl])
                    t2 = work.tile([128, 512], F32, tag="t2")
                    nc.vector.tensor_add(t2, t1, c_bc[:, sl])
                    nc.scalar.activation(o[:, sl], t2, ACTF.Identity,
                                         scale=col(g_idx + ct), bias=col(b_idx + ct))
            return outs

        # persistent pool for xnT (batch) + qT: lives until attention end
        sp1 = top.enter_context(ExitStack())
        p1 = sp1.enter_context(tc.tile_pool(name="p1", bufs=1))

        # ---------------- Phase 1: LN1 ----------------
        with ExitStack() as st:
            xnT = layernorm(xbT, T, CP_G1, CP_BL1, p1, "xnT", st)

        with ExitStack() as stq:
            xnq_pool = stq.enter_context(tc.tile_pool(name="xnqp", bufs=1))
            with ExitStack() as st:
                xnqT = layernorm(xqT, TQ, CP_G1, CP_BL1, xnq_pool, "xnqT", st)
            # ---------------- Phase 2: Q projection ----------------
            with ExitStack() as st:
                wpool = st.enter_context(tc.tile_pool(name="wq", bufs=4))
                qps = st.enter_context(tc.tile_pool(name="qps", bufs=2, space="PSUM"))
                qT = []
                for ft in range(NCT):
                    ps = qps.tile([128, TQ], F32, tag="ps")
                    for ct in range(NCT):
                        wt = wpool.tile([128, 128], F32R, tag="w")
                        nc.sync.dma_start(out=wt, in_=wq_g.ap()[ct, ft].bitcast(F32R))
                        nc.tensor.matmul(ps, wt, xnqT[ct], start=(ct == 0),
                                         stop=(ct == NCT - 1))
                    q = p1.tile([128, TQ], F32R, tag=f"qT{ft}")
                    nc.scalar.activation(q, ps, ACTF.Identity, scale=col(CP_QSCL),
                                         bias=col(CP_BQ8 + ft))
                    qT.append(q)

        # ------------- Phases 3-4: K/V + attention, two half passes -------------
        with ExitStack() as stc:
            ypool = stc.enter_context(tc.tile_pool(name="ypool", bufs=1))
            yT = []
            for ft in range(NCT):
                yt_tile = ypool.tile([128, TQ], F32, tag=f"yT{ft}")
                yT.append(yt_tile)

            for half in range(2):
                with ExitStack() as sth:
                    vpool = sth.enter_context(tc.tile_pool(name=f"v{half}", bufs=1))
                    v_sb = []
                    with ExitStack() as st:
                        wvp = st.enter_context(tc.tile_pool(name=f"wv{half}", bufs=1))
                        vps = st.enter_context(
                            tc.tile_pool(name=f"vps{half}", bufs=3, space="PSUM"))
                        wv_tiles = []
                        for ct in range(NCT):
                            wt = wvp.tile([128, 512], F32R, tag=f"wv{ct}")
                            nc.sync.dma_start(
                                out=wt,
                                in_=wv.ap()[ct * 128:(ct + 1) * 128,
                                            half * 512:half * 512 + 512].bitcast(F32R))
                            wv_tiles.append(wt)
                        for tt in range(NTT):
                            v = vpool.tile([128, 8, 65], F32R, tag=f"v{tt}")
                            nc.vector.memset(v[:, :, 64:65].bitcast(F32), 1.0)
                            v_sb.append(v)
                            ps = vps.tile([128, 512], F32, tag="ps")
                            for ct in range(NCT):
                                nc.tensor.matmul(
                                    ps, xnT[ct][:, tt * 128:(tt + 1) * 128],
                                    wv_tiles[ct],
                                    start=(ct == 0), stop=(ct == NCT - 1))
                            nc.scalar.activation(
                                v[:, :, 0:64],
                                ps.rearrange("p (h d) -> p h d", h=8), ACTF.Copy)

                    with ExitStack() as st:
                        kps = st.enter_context(
                            tc.tile_pool(name=f"kps{half}", bufs=2, space="PSUM"))
                        sps = st.enter_context(
                            tc.tile_pool(name=f"sps{half}", bufs=3, space="PSUM"))
                        bps = st.enter_context(
                            tc.tile_pool(name=f"bps{half}", bufs=1, space="PSUM"))
                        ops_ = st.enter_context(
                            tc.tile_pool(name=f"ops{half}", bufs=2, space="PSUM"))
                        kpool = st.enter_context(tc.tile_pool(name=f"k{half}", bufs=1))
                        wkp = st.enter_context(tc.tile_pool(name=f"wk{half}", bufs=1))
                        epool = st.enter_context(tc.tile_pool(name=f"e{half}", bufs=2))
                        rpool = st.enter_context(tc.tile_pool(name=f"r{half}", bufs=2))
                        for hp_local in range(4):
                            hp = half * 4 + hp_local
                            wk_tiles = []
                            for ct in range(NCT):
                                wt = wkp.tile([128, 128], F32R, tag=f"w{ct}")
                                nc.sync.dma_start(out=wt,
                                                  in_=wk_g.ap()[ct, hp].bitcast(F32R))
                                wk_tiles.append(wt)
                            kT = kpool.tile([128, T], F32R, tag=f"kT{hp_local % 2}")
                            for ch in range(T // 512):
                                ps = kps.tile([128, 512], F32, tag="ps")
                                for ct in range(NCT):
                                    nc.tensor.matmul(
                                        ps, wk_tiles[ct],
                                        xnT[ct][:, ch * 512:ch * 512 + 512],
                                        start=(ct == 0), stop=(ct == NCT - 1))
                                nc.scalar.activation(
                                    kT[:, ch * 512:ch * 512 + 512], ps,
                                    ACTF.Identity, bias=col(CP_BK + hp))
                            for hh in range(2):
                                h = hp * 2 + hh
                                p0 = 64 * hh
                                out_ps = ops_.tile([65, 512], F32, tag="out")
                                for kt in range(NTT):
                                    sc = sps.tile([128, 512], F32, tag="sc")
                                    nc.tensor.matmul(
                                        sc, kT[p0:p0 + 64, kt * 128:(kt + 1) * 128],
                                        qT[hp][p0:p0 + 64, :], start=True, stop=True)
                                    ex = epool.tile([128, 512], F32R, tag="ex")
                                    nc.scalar.activation(ex, sc, ACTF.Exp)
                                    nc.tensor.matmul(
                                        out_ps, v_sb[kt][:, h % 8, :], ex,
                                        start=(kt == 0), stop=(kt == NTT - 1))
                                rr = rpool.tile([1, 512], F32, tag="rr")
                                nc.vector.reciprocal(rr, out_ps[64:65, :])
                                rr_r = rpool.tile([1, 512], F32R, tag="rr_r")
                                nc.scalar.activation(rr_r, rr, ACTF.Copy)
                                bc = bps.tile([64, 512], F32, tag="bc")
                                nc.tensor.matmul(bc, ones_row[:, 0:64], rr_r,
                                                 start=True, stop=True)
                                bc_sb = epool.tile([64, 512], F32, tag="bcs")
                                nc.vector.tensor_copy(bc_sb, bc)
                                t1 = epool.tile([64, 512], F32, tag="yt")
                                nc.vector.tensor_mul(t1, out_ps[0:64, :], bc_sb)
                                nc.vector.tensor_scalar_add(
                                    yT[hp][p0:p0 + 64, :], t1,
                                    col(CP_BV + hp)[p0:p0 + 64, :])

            # -------- Phase 5: attention out proj + residual --------
            x2T = []
            with ExitStack() as st:
                xrp = st.enter_context(tc.tile_pool(name="xrp", bufs=1))
                wpool = st.enter_context(tc.tile_pool(name="wo", bufs=4))
                pps = st.enter_context(tc.tile_pool(name="ops2", bufs=2, space="PSUM"))
                tpool = st.enter_context(tc.tile_pool(name="t5", bufs=2))
                yT_r = []
                for ft in range(NCT):
                    r = xrp.tile([128, TQ], F32R, tag=f"yTr{ft}")
                    nc.scalar.activation(r, yT[ft], ACTF.Copy)
                    yT_r.append(r)
                xq_res = []
                for ct in range(NCT):
                    xt = xrp.tile([128, TQ], F32, tag=f"xqres{ct}")
                    nc.sync.dma_start(out=xt,
                                      in_=xqT.ap()[ct * 128:(ct + 1) * 128, :])
                    xq_res.append(xt)
                for ft in range(NCT):
                    ps = pps.tile([128, TQ], F32, tag="ps")
                    for ct in range(NCT):
                        wt = wpool.tile([128, 128], F32R, tag="w")
                        nc.sync.dma_start(out=wt, in_=wo_g.ap()[ct, ft].bitcast(F32R))
                        nc.tensor.matmul(ps, wt, yT_r[ct], start=(ct == 0),
                                         stop=(ct == NCT - 1))
                    t = tpool.tile([128, TQ], F32, tag="t")
                    nc.vector.tensor_add(t, ps, xq_res[ft])
                    x2 = p2x.tile([128, TQ], F32R, tag=f"x2T{ft}")
                    nc.scalar.activation(x2, t, ACTF.Identity, bias=col(CP_BO + ft))
                    x2T.append(x2)

        sp1.close()  # free xnT/qT/yT region before the MLP phases

        # ---------------- Phase 6-8: LN2 + MLP ----------------
        gpool = top.enter_context(tc.tile_pool(name="gpool", bufs=1))
        with ExitStack() as stg:
            hpool = stg.enter_context(tc.tile_pool(name="hpool", bufs=1))
            with ExitStack() as st:
                hT = layernorm(x2T, TQ, CP_G2, CP_BL2, hpool, "hT", st)
            gT = []
            with ExitStack() as st:
                wpool = st.enter_context(tc.tile_pool(name="w1", bufs=6))
                pps = st.enter_context(tc.tile_pool(name="m1ps", bufs=2, space="PSUM"))
                for hf in range(NHT):
                    ps = pps.tile([128, TQ], F32, tag="ps")
                    for ct in range(NCT):
                        wt = wpool.tile([128, 128], F32R, tag="w")
                        nc.sync.dma_start(out=wt, in_=w1_g.ap()[ct, hf].bitcast(F32R))
                        nc.tensor.matmul(ps, wt, hT[ct], start=(ct == 0),
                                         stop=(ct == NCT - 1))
                    g = gpool.tile([128, TQ], F32R, tag=f"gT{hf}")
                    nc.scalar.activation(g, ps, ACTF.Gelu, bias=col(CP_B1 + hf))
                    gT.append(g)

        with ExitStack() as st:
            wpool = st.enter_context(tc.tile_pool(name="w2", bufs=6))
            pps = st.enter_context(tc.tile_pool(name="m2ps", bufs=2, space="PSUM"))
            tpool = st.enter_context(tc.tile_pool(name="t8", bufs=3))
            for ft in range(NCT):
                ps = pps.tile([128, TQ], F32, tag="ps")
                for hf in range(NHT):
                    wt = wpool.tile([128, 128], F32R, tag="w")
                    nc.sync.dma_start(out=wt, in_=w2_g.ap()[hf, ft].bitcast(F32R))
                    nc.tensor.matmul(ps, wt, gT[hf], start=(hf == 0),
                                     stop=(hf == NHT - 1))
                t = tpool.tile([128, TQ], F32, tag="t")
                nc.scalar.activation(t, ps, ACTF.Identity, bias=col(CP_B2 + ft))
                o = tpool.tile([128, TQ], F32, tag="o")
                nc.vector.tensor_add(o, t, x2T[ft].bitcast(F32))
                nc.sync.dma_start(out=out_d.ap()[ft * 128:(ft + 1) * 128, :], in_=o)

    nc.compile()
    return nc


def _prep_inputs(inputs):
    x = np.asarray(inputs["x"], np.float32)
    common = dict(
        wq_g=_grid(np.asarray(inputs["Wq"], np.float32)),
        wk_g=_grid(np.asarray(inputs["Wk"], np.float32)),
        wv=np.ascontiguousarray(np.asarray(inputs["Wv"], np.float32)),
        wo_g=_grid(np.asarray(inputs["Wo"], np.float32)),
        w1_g=_grid(np.asarray(inputs["W1"], np.float32)),
        w2_g=_grid(np.asarray(inputs["W2"], np.float32)),
    )
    cpk = np.zeros((128, CP_N), np.float32)
    cpk[:, CP_BQ8:CP_BQ8 + 8] = _pack_cols(np.asarray(inputs["bq"], np.float32) * 0.125)
    cpk[:, CP_BK:CP_BK + 8] = _pack_cols(np.asarray(inputs["bk"], np.float32))
    cpk[:, CP_BO:CP_BO + 8] = _pack_cols(np.asarray(inputs["bo"], np.float32))
    cpk[:, CP_B2:CP_B2 + 8] = _pack_cols(np.asarray(inputs["b2"], np.float32))
    cpk[:, CP_G1:CP_G1 + 8] = _pack_cols(np.asarray(inputs["ln1_g"], np.float32))
    cpk[:, CP_BL1:CP_BL1 + 8] = _pack_cols(np.asarray(inputs["ln1_b"], np.float32))
    cpk[:, CP_G2:CP_G2 + 8] = _pack_cols(np.asarray(inputs["ln2_g"], np.float32))
    cpk[:, CP_BL2:CP_BL2 + 8] = _pack_cols(np.asarray(inputs["ln2_b"], np.float32))
    cpk[:, CP_BV:CP_BV + 8] = _pack_cols(np.asarray(inputs["bv"], np.float32))
    cpk[:, CP_B1:CP_B1 + 32] = _pack_cols(np.asarray(inputs["b1"], np.float32))
    cpk[:, CP_EPS] = LN_EPS
    cpk[:, CP_NEG1] = -1.0
    cpk[:, CP_QSCL] = 0.125
    common["colpack"] = cpk

    in_maps = []
    for core in range(N_CORES):
        b, s = divmod(core, N_CORES // B)
        m = dict(common)
        m["xbT"] = np.ascontiguousarray(x[b].T)
        m["xqT"] = np.ascontiguousarray(x[b, s * TQ:(s + 1) * TQ, :].T)
        in_maps.append(m)
    return in_maps


def kernel(**inputs):
    from concourse.bass_utils import run_bass_kernel_spmd
    if "nc" not in _CACHE:
        _CACHE["nc"] = _build_program()
    nc = _CACHE["nc"]
    in_maps = _prep_inputs(inputs)
    res = run_bass_kernel_spmd(nc, in_maps, list(range(N_CORES)))
    out = np.empty((B, T, C), np.float32)
    for core in range(N_CORES):
        b, s = divmod(core, N_CORES // B)
        out[b, s * TQ:(s + 1) * TQ, :] = res.results[core]["outT"].T
    return out
